# revision 21
# baseline (speedup 1.0000x reference)
"""QSP expectation kernel for Trainium2 (8 NeuronCores, data-parallel).

Math: the reference computes preds = alphas * Re(<0|U|0>) + bias where U is a
QSP chain with 55 phases. Re(<0|U|0>) as a function of theta is EXACTLY a
truncated Fourier series with period pi:

    f(theta) = c0 + sum_{k=1..27} A_k * sin(2k*theta + G_k)

(55 real degrees of freedom = 55 phases). The coefficients are recovered on
the host from 55 samples of the (cheap, 55-point) recurrence via FFT; the
spectrum decays exponentially in k, so only the first T (~12-18) harmonics
are needed.

Device kernel per core (65536 elements as [128, 512] f32), per harmonic k:
  1. DVE custom fused op (one instruction, 5 ALU stages):
        r = theta - RNE((theta + d_k)/m_k) * m_k     (m_k = pi/k)
     using the fp32 magic-number rounding trick (add/sub 1.5*2^23).
  2. ACT Sin: h = sin(r * 2k + B_k) in bf16, arg in [-pi, pi]
  3. PE  diag-matmul (bf16): acc(PSUM fp32) += A_k * h
Epilogue: preds = (acc + c0) * alphas + bias  (DVE), DMA out.
"""

import numpy as np
import ml_dtypes

import concourse.bass as bass
import concourse.tile as tile
from concourse import mybir as mb
import concourse.bass_utils as _bu
from concourse.bass_utils import run_bass_kernel_spmd

QSP_DEPTH = 27
N_PHIS = 2 * QSP_DEPTH + 1  # 55
B = 524288
N_CORES = 8
P = 128
F = B // N_CORES // P  # 512
REL_TOL_TARGET = 5e-3  # truncation budget (harness gate is 2e-2)
T_MIN, T_MAX = 6, 27
MAGIC = float(np.float32(1.5 * 2.0**23))  # fp32 RNE-to-integer constant

# The walrus NEFF epilogue clears its semaphore space one-at-a-time; cap the
# allocator (harmless if ignored).
if not getattr(_bu, "_max_sem_patched", False):
    _orig_get_walrus_args = _bu.get_walrus_args

    def _patched_get_walrus_args(*a, **kw):
        return ["--max-sem-num=32", *_orig_get_walrus_args(*a, **kw)]

    _bu.get_walrus_args = _patched_get_walrus_args
    _bu._max_sem_patched = True


# ---------------------------------------------------------------------------
# host-side math
# ---------------------------------------------------------------------------

def _qsp_scalar(theta, phis):
    theta = np.asarray(theta, dtype=np.float64)
    phis = np.asarray(phis, dtype=np.float64)
    c, s = np.cos(theta), np.sin(theta)
    r0r = np.ones_like(theta)
    r0i = np.zeros_like(theta)
    r1r = np.zeros_like(theta)
    r1i = np.zeros_like(theta)
    for phi in phis[1:]:
        cp, sp = np.cos(phi), np.sin(phi)
        ar = r0r * c - r1i * s
        ai = r0i * c + r1r * s
        br = r1r * c - r0i * s
        bi = r1i * c + r0r * s
        r0r = ar * cp - ai * sp
        r0i = ar * sp + ai * cp
        r1r = br * cp + bi * sp
        r1i = bi * cp - br * sp
    return r0r * np.cos(phis[0]) - r0i * np.sin(phis[0])


def _fourier_coeffs(phis):
    j = np.arange(N_PHIS)
    theta_j = np.pi * j / N_PHIS
    C = np.fft.fft(_qsp_scalar(theta_j, phis)) / N_PHIS
    c0 = float(np.real(C[0]))
    A = 2.0 * np.abs(C[1 : QSP_DEPTH + 1])
    G = np.angle(C[1 : QSP_DEPTH + 1]) + np.pi / 2
    return c0, A, G


def _choose_T(c0, A):
    meanp = max(c0 * c0 + float((A**2).sum()) / 2.0, 1e-12)
    for T in range(T_MIN, T_MAX):
        tail = float((A[T:] ** 2).sum()) / 2.0
        if np.sqrt(tail / meanp) < REL_TOL_TARGET:
            return T
    return T_MAX


# ---------------------------------------------------------------------------
# device program
# ---------------------------------------------------------------------------

def _split_excess_waits(nc):
    """This walrus build only supports ONE sem-wait per instruction; spill
    extra waits onto preceding same-engine NOPs."""
    cnt = 0
    for bb in nc.main_func.blocks:
        out, changed = [], False
        for ins in list(bb.instructions):
            si = ins.sync_info
            if si is not None and len(si.on_wait) > 1:
                waits = list(si.on_wait)
                for w in waits[1:]:
                    nop = mb.InstNoOp(name=f"waitsplit-{cnt}", ins=[], outs=[])
                    cnt += 1
                    nop.engine = ins.engine
                    nop.sync_info = mb.SyncInfo(on_wait=[w], on_update=[])
                    nc.register_instruction(nop)
                    out.append(nop)
                ins.sync_info = mb.SyncInfo(on_wait=waits[:1], on_update=list(si.on_update))
                changed = True
            out.append(ins)
        if changed:
            bb.instructions = out
    return cnt


def _strip_const_init(nc):
    """Remove the unused const-AP memsets + the init all-engine barrier
    (saves ~1us of head; the barrier protocol is self-balancing so removing
    a complete barrier is protocol-neutral)."""
    bb0 = nc.main_func.blocks[0]
    il = list(bb0.instructions)
    # find the const memsets
    ms_idx = [i for i, ins in enumerate(il)
              if ins.__class__.__name__ == "InstMemset"
              and ins.outs and "const-" in str(getattr(ins.outs[0], "memref", ""))]
    if not ms_idx:
        return 0
    start = min(ms_idx)
    # drop memsets and the following Drain/EventSemaphore barrier group until
    # the first branch
    drop = set(ms_idx)
    for i in range(max(ms_idx) + 1, len(il)):
        cn = il[i].__class__.__name__
        if cn in ("InstDrain", "InstEventSemaphore"):
            drop.add(i)
        elif cn == "InstUnconditionalBranch":
            break
        else:
            break
    bb0.instructions = [ins for i, ins in enumerate(il) if i not in drop]
    return len(drop)




def _strip_exit_tail(nc):
    """Drop the Tile-exit Pool sem range-clear + the SECOND exit all-engine
    barrier: the walrus NEFF epilogue clears the entire sem file per engine
    anyway, so both are redundant. Barrier #1 (global rendezvous) stays —
    it prevents an engine's epilogue clear racing another engine's waits."""
    bb = nc.main_func.blocks[-1]
    il = list(bb.instructions)
    # find the Pool InstISA (dma_reset/sem_clear group); drop it, its guard
    # drains, and everything after (the second barrier group)
    isa_idx = [i for i, ins in enumerate(il) if ins.__class__.__name__ == "InstISA"]
    if not isa_idx:
        return 0
    cut = isa_idx[0]
    # keep instructions before the Pool drain that precedes the ISA
    while cut > 0 and il[cut - 1].__class__.__name__ == "InstDrain" and str(il[cut - 1].engine).endswith("Pool"):
        cut -= 1
    dropped = len(il) - cut
    bb.instructions = il[:cut]
    return dropped



def _strip_exit_waits(nc):
    """The Tile exit rendezvous makes SP wait on EVERY proc's final sem tick.
    Transitively, the output DMA already depends on the whole compute chain;
    only its completion sem must gate the exit barrier. Drop the rest."""
    out_sem = None
    for bb in nc.main_func.blocks:
        for ins in bb.instructions:
            if ins.__class__.__name__ == "InstDMACopy" and ins.sync_info and ins.sync_info.on_update:
                out_sem = ins.sync_info.on_update[0].id
    if out_sem is None:
        return 0
    bb = nc.main_func.blocks[-1]
    il = list(bb.instructions)
    keep, dropped = [], 0
    for ins in il:
        if (ins.__class__.__name__ == "InstNoOp"
                and str(getattr(ins, "name", "")).startswith("waitsplit")
                and ins.sync_info and len(ins.sync_info.on_wait) == 1
                and ins.sync_info.on_wait[0].id != out_sem
                and str(ins.engine).endswith("SP")):
            dropped += 1
            continue
        keep.append(ins)
    bb.instructions = keep
    return dropped

# params columns per harmonic i (k=i+1):
#   4i+0: 1/m_k            (s0 of custom op)
#   4i+1: delta/m + MAGIC  (s1 of custom op)
#   4i+2: m_k              (in1 of custom op, latched scalar)
#   4i+3: B_k = 2k*delta   (ACT bias)
# then: [4T] = c0, [4T+1] = bias
_NC_CACHE = {}


def _build_nc(T, with_bias, delta):
    key = (T, with_bias, tuple(np.float32(delta[:T])))
    if key in _NC_CACHE:
        return _NC_CACHE[key]

    nc = bass.Bass("TRN2", target_bir_lowering=False, debug=False, num_devices=N_CORES)
    f32 = mb.dt.float32

    th_d = nc.dram_tensor("theta", [P, F], f32, kind="ExternalInput")
    al_d = nc.dram_tensor("alphas", [P, F], f32, kind="ExternalInput")
    par_d = nc.dram_tensor("params", [P, 4 * T + 2], f32, kind="ExternalInput")
    dg_d = nc.dram_tensor("diags", [P, T * P], mb.dt.bfloat16, kind="ExternalInput")
    out_d = nc.dram_tensor("preds", [P, F], f32, kind="ExternalOutput")

    with tile.TileContext(nc) as tc:
        with (
            tc.tile_pool(name="persist", bufs=1) as persist,
            tc.tile_pool(name="work", bufs=1) as work,
            tc.tile_pool(name="psum", bufs=1, space="PSUM") as psum,
        ):
            # tiny ring-warmers: absorb HWDGE first-transfer latency so the
            # theta halves ride warm queues
            wa = persist.tile([1, 8], f32, tag="wa")
            nc.scalar.dma_start(wa[:], th_d.ap()[0:1, 0:8])
            wb = persist.tile([1, 8], f32, tag="wb")
            nc.sync.dma_start(wb[:], th_d.ap()[0:1, 0:8])

            th = persist.tile([P, F], f32, tag="th")
            nc.scalar.dma_start(th[0:64, :], th_d.ap()[0:64, :])
            nc.sync.dma_start(th[64:128, :], th_d.ap()[64:128, :])

            # preload the Sin activation table while input DMAs run.
            # warm is read uninitialized on purpose: only the ACT_TABLE_LOAD
            # side effect matters (output unused; HW clamps any garbage), and
            # a memset would chain this behind slow-booting GPSIMD.
            warm = persist.tile([P, 1], f32, tag="warm")
            nc.scalar.activation(warm[:], warm[:], mb.ActivationFunctionType.Sin,
                                 bias=warm[:, 0:1], scale=0.0)
            par = persist.tile([P, 4 * T + 2], f32, tag="par")
            nc.sync.dma_start(par[:], par_d.ap())
            dg = persist.tile([P, T * P], mb.dt.bfloat16, tag="dg")
            nc.sync.dma_start(dg[:], dg_d.ap())
            al = persist.tile([P, F], f32, tag="al")
            nc.sync.dma_start(al[:], al_d.ap())

            acc = psum.tile([P, F], f32, tag="acc")

            for i in range(T):
                k = i + 1
                m_k = float(np.float32(np.pi / k))
                n_t = work.tile([P, F], mb.dt.int16, tag=f"n{i}")
                # n = int16_rne((theta + delta) * (1/m))
                nc.vector.tensor_scalar(
                    n_t[:], th[:], float(delta[i]), float(1.0 / (np.pi / k)),
                    mb.AluOpType.add, mb.AluOpType.mult,
                )
                r_t = work.tile([P, F], f32, tag=f"r{i}")
                nc.vector.scalar_tensor_tensor(
                    r_t[:], n_t[:], -m_k, th[:],
                    mb.AluOpType.mult, mb.AluOpType.add,
                )
                h_t = work.tile([P, F], mb.dt.bfloat16, tag=f"h{i}")
                nc.scalar.activation(
                    h_t[:], r_t[:], mb.ActivationFunctionType.Sin,
                    bias=par[:, 4 * i + 3 : 4 * i + 4], scale=float(2.0 * k),
                )
                nc.tensor.matmul(
                    acc[:], dg[:, i * P : (i + 1) * P], h_t[:],
                    start=(i == 0), stop=(i == T - 1),
                )

            # preds = (acc + c0) * alphas + bias
            tmp = work.tile([P, F], f32, tag="tmp")
            nc.vector.scalar_tensor_tensor(
                tmp[:], acc[:], par[:, 4 * T : 4 * T + 1], al[:],
                mb.AluOpType.add, mb.AluOpType.mult,
            )
            if with_bias:
                pred_t = work.tile([P, F], f32, tag="pred")
                nc.vector.tensor_scalar(
                    pred_t[:], tmp[:], par[:, 4 * T + 1 : 4 * T + 2], None,
                    mb.AluOpType.add,
                )
                nc.sync.dma_start(out_d.ap(), pred_t[:])
            else:
                nc.sync.dma_start(out_d.ap(), tmp[:])

    _split_excess_waits(nc)
    _strip_exit_tail(nc)
    _strip_exit_waits(nc)
    _NC_CACHE[key] = nc
    return nc


def _prepare(x, qsp_params, alphas, bias):
    """Host preprocessing: Fourier coefficients -> per-harmonic constants,
    per-core input maps. Returns (nc, in_maps)."""
    x = np.asarray(x, dtype=np.float32)
    qsp_params = np.asarray(qsp_params, dtype=np.float64)
    alphas = np.asarray(alphas, dtype=np.float32)
    bias_v = float(np.asarray(bias, dtype=np.float64).reshape(-1)[0])

    c0, A, G = _fourier_coeffs(qsp_params)
    T = _choose_T(c0, A)

    ks = np.arange(1, T + 1, dtype=np.float64)
    m = np.pi / ks
    delta = np.mod(G[:T], 2 * np.pi) / (2 * ks)  # in [0, m)
    Bact = 2 * ks * delta  # ACT bias; arg = 2k*r + Bact in [-pi-eps, pi+eps]

    params = np.zeros((P, 4 * T + 2), dtype=np.float32)
    for i in range(T):
        params[:, 4 * i] = delta[i]
        params[:, 4 * i + 1] = 0.0
        params[:, 4 * i + 2] = m[i]
        params[:, 4 * i + 3] = Bact[i]
    params[:, 4 * T] = c0
    params[:, 4 * T + 1] = bias_v

    eye = np.eye(P, dtype=np.float32)
    diags = np.ascontiguousarray(
        np.concatenate([A[i] * eye for i in range(T)], axis=1).astype(ml_dtypes.bfloat16)
    )

    theta = x[:, 0]
    per_core = B // N_CORES
    in_maps = []
    for c in range(N_CORES):
        sl = slice(c * per_core, (c + 1) * per_core)
        in_maps.append(
            {
                "theta": np.ascontiguousarray(theta[sl].reshape(P, F)),
                "alphas": np.ascontiguousarray(alphas[sl].reshape(P, F)),
                "params": params,
                "diags": diags,
            }
        )
    return _build_nc(T, bias_v != 0.0, delta), in_maps


def kernel(x, qsp_params, alphas, bias):
    nc, in_maps = _prepare(x, qsp_params, alphas, bias)
    res = run_bass_kernel_spmd(nc, in_maps, core_ids=list(range(N_CORES)))
    per_core = B // N_CORES
    out = np.empty((B,), dtype=np.float32)
    for c in range(N_CORES):
        out[c * per_core : (c + 1) * per_core] = res.results[c]["preds"].reshape(-1)
    return out[:, None]


# revision 22
# speedup vs baseline: 1.1023x; 1.1023x over previous
"""QSP expectation kernel for Trainium2 (8 NeuronCores, data-parallel).

Math: the reference computes preds = alphas * Re(<0|U|0>) + bias where U is a
QSP chain with 55 phases. Re(<0|U|0>) as a function of theta is EXACTLY a
truncated Fourier series with period pi:

    f(theta) = c0 + sum_{k=1..27} A_k * sin(2k*theta + G_k)

(55 real degrees of freedom = 55 phases). The coefficients are recovered on
the host from 55 samples of the (cheap, 55-point) recurrence via FFT; the
spectrum decays exponentially in k, so only the first T (~12-18) harmonics
are needed.

Device kernel per core (65536 elements as [128, 512] f32), per harmonic k:
  1. DVE custom fused op (one instruction, 5 ALU stages):
        r = theta - RNE((theta + d_k)/m_k) * m_k     (m_k = pi/k)
     using the fp32 magic-number rounding trick (add/sub 1.5*2^23).
  2. ACT Sin: h = sin(r * 2k + B_k) in bf16, arg in [-pi, pi]
  3. PE  diag-matmul (bf16): acc(PSUM fp32) += A_k * h
Epilogue: preds = (acc + c0) * alphas + bias  (DVE), DMA out.
"""

import numpy as np
import ml_dtypes

import concourse.bass as bass
import concourse.tile as tile
from concourse import mybir as mb
import concourse.bass_utils as _bu
from concourse.bass_utils import run_bass_kernel_spmd

QSP_DEPTH = 27
N_PHIS = 2 * QSP_DEPTH + 1  # 55
B = 524288
N_CORES = 8
P = 128
F = B // N_CORES // P  # 512
REL_TOL_TARGET = 5e-3  # truncation budget (harness gate is 2e-2)
T_MIN, T_MAX = 6, 27
MAGIC = float(np.float32(1.5 * 2.0**23))  # fp32 RNE-to-integer constant

# The walrus NEFF epilogue clears its semaphore space one-at-a-time; cap the
# allocator (harmless if ignored).
if not getattr(_bu, "_max_sem_patched", False):
    _orig_get_walrus_args = _bu.get_walrus_args

    def _patched_get_walrus_args(*a, **kw):
        return ["--max-sem-num=32", *_orig_get_walrus_args(*a, **kw)]

    _bu.get_walrus_args = _patched_get_walrus_args
    _bu._max_sem_patched = True


# ---------------------------------------------------------------------------
# host-side math
# ---------------------------------------------------------------------------

def _qsp_scalar(theta, phis):
    theta = np.asarray(theta, dtype=np.float64)
    phis = np.asarray(phis, dtype=np.float64)
    c, s = np.cos(theta), np.sin(theta)
    r0r = np.ones_like(theta)
    r0i = np.zeros_like(theta)
    r1r = np.zeros_like(theta)
    r1i = np.zeros_like(theta)
    for phi in phis[1:]:
        cp, sp = np.cos(phi), np.sin(phi)
        ar = r0r * c - r1i * s
        ai = r0i * c + r1r * s
        br = r1r * c - r0i * s
        bi = r1i * c + r0r * s
        r0r = ar * cp - ai * sp
        r0i = ar * sp + ai * cp
        r1r = br * cp + bi * sp
        r1i = bi * cp - br * sp
    return r0r * np.cos(phis[0]) - r0i * np.sin(phis[0])


def _fourier_coeffs(phis):
    j = np.arange(N_PHIS)
    theta_j = np.pi * j / N_PHIS
    C = np.fft.fft(_qsp_scalar(theta_j, phis)) / N_PHIS
    c0 = float(np.real(C[0]))
    A = 2.0 * np.abs(C[1 : QSP_DEPTH + 1])
    G = np.angle(C[1 : QSP_DEPTH + 1]) + np.pi / 2
    return c0, A, G


def _choose_T(c0, A):
    meanp = max(c0 * c0 + float((A**2).sum()) / 2.0, 1e-12)
    for T in range(T_MIN, T_MAX):
        tail = float((A[T:] ** 2).sum()) / 2.0
        if np.sqrt(tail / meanp) < REL_TOL_TARGET:
            return T
    return T_MAX


# ---------------------------------------------------------------------------
# device program
# ---------------------------------------------------------------------------

def _split_excess_waits(nc):
    """This walrus build only supports ONE sem-wait per instruction; spill
    extra waits onto preceding same-engine NOPs."""
    cnt = 0
    for bb in nc.main_func.blocks:
        out, changed = [], False
        for ins in list(bb.instructions):
            si = ins.sync_info
            if si is not None and len(si.on_wait) > 1:
                waits = list(si.on_wait)
                for w in waits[1:]:
                    nop = mb.InstNoOp(name=f"waitsplit-{cnt}", ins=[], outs=[])
                    cnt += 1
                    nop.engine = ins.engine
                    nop.sync_info = mb.SyncInfo(on_wait=[w], on_update=[])
                    nc.register_instruction(nop)
                    out.append(nop)
                ins.sync_info = mb.SyncInfo(on_wait=waits[:1], on_update=list(si.on_update))
                changed = True
            out.append(ins)
        if changed:
            bb.instructions = out
    return cnt


def _strip_const_init(nc):
    """Remove the unused const-AP memsets + the init all-engine barrier
    (saves ~1us of head; the barrier protocol is self-balancing so removing
    a complete barrier is protocol-neutral)."""
    bb0 = nc.main_func.blocks[0]
    il = list(bb0.instructions)
    # find the const memsets
    ms_idx = [i for i, ins in enumerate(il)
              if ins.__class__.__name__ == "InstMemset"
              and ins.outs and "const-" in str(getattr(ins.outs[0], "memref", ""))]
    if not ms_idx:
        return 0
    start = min(ms_idx)
    # drop memsets and the following Drain/EventSemaphore barrier group until
    # the first branch
    drop = set(ms_idx)
    for i in range(max(ms_idx) + 1, len(il)):
        cn = il[i].__class__.__name__
        if cn in ("InstDrain", "InstEventSemaphore"):
            drop.add(i)
        elif cn == "InstUnconditionalBranch":
            break
        else:
            break
    bb0.instructions = [ins for i, ins in enumerate(il) if i not in drop]
    return len(drop)




def _strip_exit_tail(nc):
    """Drop the Tile-exit Pool sem range-clear + the SECOND exit all-engine
    barrier: the walrus NEFF epilogue clears the entire sem file per engine
    anyway, so both are redundant. Barrier #1 (global rendezvous) stays —
    it prevents an engine's epilogue clear racing another engine's waits."""
    bb = nc.main_func.blocks[-1]
    il = list(bb.instructions)
    # find the Pool InstISA (dma_reset/sem_clear group); drop it, its guard
    # drains, and everything after (the second barrier group)
    isa_idx = [i for i, ins in enumerate(il) if ins.__class__.__name__ == "InstISA"]
    if not isa_idx:
        return 0
    cut = isa_idx[0]
    # keep instructions before the Pool drain that precedes the ISA
    while cut > 0 and il[cut - 1].__class__.__name__ == "InstDrain" and str(il[cut - 1].engine).endswith("Pool"):
        cut -= 1
    dropped = len(il) - cut
    bb.instructions = il[:cut]
    return dropped



def _strip_exit_waits(nc):
    """The Tile exit rendezvous makes SP wait on EVERY proc's final sem tick.
    Transitively, the output DMA already depends on the whole compute chain;
    only its completion sem must gate the exit barrier. Drop the rest."""
    out_sem = None
    for bb in nc.main_func.blocks:
        for ins in bb.instructions:
            if ins.__class__.__name__ == "InstDMACopy" and ins.sync_info and ins.sync_info.on_update:
                out_sem = ins.sync_info.on_update[0].id
    if out_sem is None:
        return 0
    bb = nc.main_func.blocks[-1]
    il = list(bb.instructions)
    keep, dropped = [], 0
    for ins in il:
        if (ins.__class__.__name__ == "InstNoOp"
                and str(getattr(ins, "name", "")).startswith("waitsplit")
                and ins.sync_info and len(ins.sync_info.on_wait) == 1
                and ins.sync_info.on_wait[0].id != out_sem
                and str(ins.engine).endswith("SP")):
            dropped += 1
            continue
        keep.append(ins)
    bb.instructions = keep
    return dropped

# params columns per harmonic i (k=i+1):
#   4i+0: 1/m_k            (s0 of custom op)
#   4i+1: delta/m + MAGIC  (s1 of custom op)
#   4i+2: m_k              (in1 of custom op, latched scalar)
#   4i+3: B_k = 2k*delta   (ACT bias)
# then: [4T] = c0, [4T+1] = bias
_NC_CACHE = {}


def _build_nc(T, with_bias, delta):
    key = (T, with_bias, tuple(np.float32(delta[:T])))
    if key in _NC_CACHE:
        return _NC_CACHE[key]

    nc = bass.Bass("TRN2", target_bir_lowering=False, debug=False, num_devices=N_CORES)
    f32 = mb.dt.float32

    th_d = nc.dram_tensor("theta", [P, F], f32, kind="ExternalInput")
    al_d = nc.dram_tensor("alphas", [P, F], f32, kind="ExternalInput")
    par_d = nc.dram_tensor("params", [P, 4 * T + 2], f32, kind="ExternalInput")
    dg_d = nc.dram_tensor("diags", [P, T * P], mb.dt.bfloat16, kind="ExternalInput")
    out_d = nc.dram_tensor("preds", [P, F], f32, kind="ExternalOutput")

    with tile.TileContext(nc) as tc:
        with (
            tc.tile_pool(name="persist", bufs=1) as persist,
            tc.tile_pool(name="work", bufs=1) as work,
            tc.tile_pool(name="psum", bufs=1, space="PSUM") as psum,
        ):
            th = persist.tile([P, F], f32, tag="th")
            nc.scalar.dma_start(th[0:64, :], th_d.ap()[0:64, :])
            nc.sync.dma_start(th[64:128, :], th_d.ap()[64:128, :])

            # preload the Sin activation table while input DMAs run.
            # warm is read uninitialized on purpose: only the ACT_TABLE_LOAD
            # side effect matters (output unused; HW clamps any garbage), and
            # a memset would chain this behind slow-booting GPSIMD.
            warm = persist.tile([P, 1], f32, tag="warm")
            nc.scalar.activation(warm[:], warm[:], mb.ActivationFunctionType.Sin,
                                 bias=warm[:, 0:1], scale=0.0)
            par = persist.tile([P, 4 * T + 2], f32, tag="par")
            nc.sync.dma_start(par[:], par_d.ap())
            dg = persist.tile([P, T * P], mb.dt.bfloat16, tag="dg")
            nc.sync.dma_start(dg[:], dg_d.ap())
            al = persist.tile([P, F], f32, tag="al")
            nc.sync.dma_start(al[:], al_d.ap())

            acc = psum.tile([P, F], f32, tag="acc")

            for i in range(T):
                k = i + 1
                m_k = float(np.float32(np.pi / k))
                n_t = work.tile([P, F], mb.dt.int16, tag=f"n{i}")
                # n = int16_rne((theta + delta) * (1/m))
                nc.vector.tensor_scalar(
                    n_t[:], th[:], float(delta[i]), float(1.0 / (np.pi / k)),
                    mb.AluOpType.add, mb.AluOpType.mult,
                )
                r_t = work.tile([P, F], f32, tag=f"r{i}")
                nc.vector.scalar_tensor_tensor(
                    r_t[:], n_t[:], -m_k, th[:],
                    mb.AluOpType.mult, mb.AluOpType.add,
                )
                h_t = work.tile([P, F], mb.dt.bfloat16, tag=f"h{i}")
                nc.scalar.activation(
                    h_t[:], r_t[:], mb.ActivationFunctionType.Sin,
                    bias=par[:, 4 * i + 3 : 4 * i + 4], scale=float(2.0 * k),
                )
                nc.tensor.matmul(
                    acc[:], dg[:, i * P : (i + 1) * P], h_t[:],
                    start=(i == 0), stop=(i == T - 1),
                )

            # preds = (acc + c0) * alphas + bias
            tmp = work.tile([P, F], f32, tag="tmp")
            nc.vector.scalar_tensor_tensor(
                tmp[:], acc[:], par[:, 4 * T : 4 * T + 1], al[:],
                mb.AluOpType.add, mb.AluOpType.mult,
            )
            if with_bias:
                pred_t = work.tile([P, F], f32, tag="pred")
                nc.vector.tensor_scalar(
                    pred_t[:], tmp[:], par[:, 4 * T + 1 : 4 * T + 2], None,
                    mb.AluOpType.add,
                )
                nc.sync.dma_start(out_d.ap(), pred_t[:])
            else:
                nc.sync.dma_start(out_d.ap(), tmp[:])

    _split_excess_waits(nc)
    _strip_exit_tail(nc)
    _strip_exit_waits(nc)
    _NC_CACHE[key] = nc
    return nc


def _prepare(x, qsp_params, alphas, bias):
    """Host preprocessing: Fourier coefficients -> per-harmonic constants,
    per-core input maps. Returns (nc, in_maps)."""
    x = np.asarray(x, dtype=np.float32)
    qsp_params = np.asarray(qsp_params, dtype=np.float64)
    alphas = np.asarray(alphas, dtype=np.float32)
    bias_v = float(np.asarray(bias, dtype=np.float64).reshape(-1)[0])

    c0, A, G = _fourier_coeffs(qsp_params)
    T = _choose_T(c0, A)

    ks = np.arange(1, T + 1, dtype=np.float64)
    m = np.pi / ks
    delta = np.mod(G[:T], 2 * np.pi) / (2 * ks)  # in [0, m)
    Bact = 2 * ks * delta  # ACT bias; arg = 2k*r + Bact in [-pi-eps, pi+eps]

    params = np.zeros((P, 4 * T + 2), dtype=np.float32)
    for i in range(T):
        params[:, 4 * i] = delta[i]
        params[:, 4 * i + 1] = 0.0
        params[:, 4 * i + 2] = m[i]
        params[:, 4 * i + 3] = Bact[i]
    params[:, 4 * T] = c0
    params[:, 4 * T + 1] = bias_v

    eye = np.eye(P, dtype=np.float32)
    diags = np.ascontiguousarray(
        np.concatenate([A[i] * eye for i in range(T)], axis=1).astype(ml_dtypes.bfloat16)
    )

    theta = x[:, 0]
    per_core = B // N_CORES
    in_maps = []
    for c in range(N_CORES):
        sl = slice(c * per_core, (c + 1) * per_core)
        in_maps.append(
            {
                "theta": np.ascontiguousarray(theta[sl].reshape(P, F)),
                "alphas": np.ascontiguousarray(alphas[sl].reshape(P, F)),
                "params": params,
                "diags": diags,
            }
        )
    return _build_nc(T, bias_v != 0.0, delta), in_maps


def kernel(x, qsp_params, alphas, bias):
    nc, in_maps = _prepare(x, qsp_params, alphas, bias)
    res = run_bass_kernel_spmd(nc, in_maps, core_ids=list(range(N_CORES)))
    per_core = B // N_CORES
    out = np.empty((B,), dtype=np.float32)
    for c in range(N_CORES):
        out[c * per_core : (c + 1) * per_core] = res.results[c]["preds"].reshape(-1)
    return out[:, None]
